# revision 6
# baseline (speedup 1.0000x reference)
"""ARMA GNN (2-layer, K=2 stacks) Trainium2 kernel.

Strategy (8-core SPMD, node-sharded), v8 "host-built fp8 S + chunk-split L2":
  * norm folding: norm[e] = dinv[row]*dinv[col]; aggregation is linear, so
    each layer aggregates RAW scaled features and applies the weight matrix
    per 128-node window afterwards:
        xagg[n] = sum_{e: col=n} (dinv[row]*feat[row])
        out_k   = relu(0.5*(dinv[n]*(xagg @ Wk) + feat@RWk + bk)); mean_k
  * The per-block selection matrices S (one-hot edge->target-slot, with
    per-window source dedup folded in as multiplicities) are built on the
    HOST in fp8 and streamed from HBM on the HWDGE path, so the DVE does no
    is_equal work at all.  Both layers share the token/S streams.
  * Gather-table rows live in a chunk-major permuted row space; the A/B
    half split (int16 gather-index range) is aligned to collective-chunk
    boundaries (B0 = first two chunks).  Layer 2 runs as two passes:
    pass-A gathers rows < B0 (available as soon as cc chunks 0-1 land,
    i.e. while layer 1 is still running) and banks aggA.T per window in
    SBUF fp16; pass-B gathers the rest after the cc tail and combines
    both halves in the window transform (two accumulating matmuls).
  * root2 is built from local h1 interleaved into pass-A; no serial mid
    phase.  Layer-1 epilogue writes dinv*h1 into the low half of 128-wide
    padded ccpad rows; chunked AllGathers are issued mid-layer-1.
  * Flipped aggregation matmul: lhsT = gathered 128-edge block M[e,f]
    (stationary), rhs = S[e,slot] streamed fp8 -> PSUM agg.T[f,slot].
  * Edges sharded by target; dma_gather on 4 SWDGE queues round-robin.

kernel(**inputs) takes the FULL problem inputs and returns the FULL output.
"""

import sys

sys.path.insert(0, "/opt/trn_rl_repo")

from contextlib import ExitStack

import numpy as np

P = 128


class Cfg:
    def __init__(self, N, NC, SHARD, B0, WB=2, SUPER=4,
                 FIN=128, HID=64, FOUT=64, K=2, SP=False, SDT="f8"):
        self.N, self.NC, self.SHARD, self.B0 = N, NC, SHARD, B0
        self.WB, self.SUPER = WB, SUPER
        self.FIN, self.HID, self.FOUT, self.K = FIN, HID, FOUT, K
        self.SP, self.SDT = SP, SDT
        self.NSTAR = NC * SHARD
        self.W = SHARD // P
        self.HALFA = B0
        self.HALFB = self.NSTAR - B0
        assert B0 % (NC * P) == 0 and SHARD % P == 0
        assert self.HALFA <= 32768 and self.HALFB <= 32768
        assert N > B0 and N < self.NSTAR
        assert K * HID == 128 and K * FOUT == 128 and FIN == 128
        self.wA = B0 // (NC * P)
        self.wB = self.W - self.wA
        self.batches = [tuple(range(b, min(b + WB, self.W)))
                        for b in range(0, self.W, WB)]
        self.sbatches = [self.batches[i:i + SUPER]
                         for i in range(0, len(self.batches), SUPER)]

        def split(n, fracs):
            sizes, rem = [], n
            for f in fracs:
                if rem <= 0:
                    break
                s = max(1, min(rem, round(n * f)))
                sizes.append(s)
                rem -= s
            if rem:
                sizes.append(rem)
            return sizes

        # chunk boundary between A-chunks and B-chunks is exactly wA
        sizes = split(self.wA, (0.55,)) + split(self.wB, (0.5, 0.25, 0.15))
        self.cc_chunks = []
        w0 = 0
        for sz in sizes:
            self.cc_chunks.append((w0, w0 + sz))
            w0 += sz
        assert w0 == self.W
        assert any(w1 == self.wA for (_, w1) in self.cc_chunks)
        self.chunk_base = []
        b = 0
        for (w0, w1) in self.cc_chunks:
            self.chunk_base.append(b)
            b += NC * (w1 - w0) * P
        assert b == self.NSTAR

    def perm(self, node):
        """node-id -> permuted gather-table row (static, data-independent)."""
        node = np.asarray(node)
        r, l = node // self.SHARD, node % self.SHARD
        w = l // P
        prow = np.zeros_like(node)
        for q, (w0, w1) in enumerate(self.cc_chunks):
            m = (w >= w0) & (w < w1)
            prow = np.where(
                m, self.chunk_base[q] + r * (w1 - w0) * P + (l - w0 * P), prow)
        return prow


REAL = dict(N=50000, NC=8, SHARD=6272, B0=29696)


def _s_np_dtype(sdt):
    if sdt == "f8":
        import ml_dtypes
        return ml_dtypes.float8_e4m3fn
    return np.float16


# --------------------------------------------------------------------------
# host preprocessing
# --------------------------------------------------------------------------
def _preprocess(c: Cfg, x, edge_index, init_w1, root_w1, b1, init_w2, root_w2, b2):
    N, NC, SHARD = c.N, c.NC, c.SHARD
    row = np.asarray(edge_index[0]).astype(np.int64)
    col = np.asarray(edge_index[1]).astype(np.int64)
    x = np.asarray(x, dtype=np.float32)

    deg = np.bincount(col, minlength=N).astype(np.float64)
    dinv = np.where(deg > 0, deg ** -0.5, 0.0).astype(np.float32)
    dinv_full = np.zeros(c.NSTAR, np.float32)
    dinv_full[:N] = dinv

    prow_all = c.perm(row)

    # pass 1: per-core sorted+deduped streams and unique counts
    percore = []
    cntA = np.zeros((NC, c.W), np.int64)
    cntB = np.zeros((NC, c.W), np.int64)
    for cc in range(NC):
        base = cc * SHARD
        m = (col >= base) & (col < base + SHARD)
        ec = (col[m] - base).astype(np.int64)
        es = prow_all[m]
        half = (es >= c.B0).astype(np.int64)
        key = (ec >> 7) * 2 + half
        order = np.lexsort((es, key))
        ek, ee, ecol = key[order], es[order], (ec & 127)[order]
        new_group = np.empty(len(ek), bool)
        if len(ek):
            new_group[0] = True
            new_group[1:] = (ek[1:] != ek[:-1]) | (ee[1:] != ee[:-1])
        uid = np.cumsum(new_group) - 1
        u_key = ek[new_group]
        u_es = ee[new_group]
        gstart = np.searchsorted(u_key, np.arange(2 * c.W + 1))
        posu = uid - gstart[ek]
        cnt = gstart[1:] - gstart[:-1]
        cntA[cc] = cnt[0::2]
        cntB[cc] = cnt[1::2]
        percore.append((ek, ecol, posu, u_key, u_es, gstart))

    NBA = [max(1, int(-(-cntA[:, w].max() // P))) for w in range(c.W)]
    NBB = [max(1, int(-(-cntB[:, w].max() // P))) for w in range(c.W)]
    NBAtot, NBBtot = sum(NBA), sum(NBB)
    blkA_base = np.concatenate([[0], np.cumsum(NBA)]).astype(np.int64)
    blkB_base = np.concatenate([[0], np.cumsum(NBB)]).astype(np.int64)
    sdt = _s_np_dtype(c.SDT)

    def build_core(cc):
        ek, ecol, posu, u_key, u_es, gstart = percore[cc]
        w_e, h_e = ek >> 1, ek & 1
        # S matrices, streamed block-major per stream
        outs = []
        for h, NB_base, NBtot in ((0, blkA_base, NBAtot), (1, blkB_base, NBBtot)):
            sel = h_e == h
            g = NB_base[w_e[sel]] + posu[sel] // P
            part = posu[sel] % P
            flat = part * (NBtot * P) + g * P + ecol[sel]
            S = np.bincount(flat, minlength=P * NBtot * P).astype(np.float32)
            outs.append(np.ascontiguousarray(
                S.reshape(P, NBtot * P).astype(sdt)))
        # token streams padded per (window, half)
        toksA, toksB = [], []
        for w in range(c.W):
            for h, toks, NB in ((0, toksA, NBA), (1, toksB, NBB)):
                gi = 2 * w + h
                lo, hi = gstart[gi], gstart[gi + 1]
                k = hi - lo
                t = np.zeros(NB[w] * P, np.int64)
                t[:k] = u_es[lo:hi] - (c.B0 if h else 0)
                toks.append(t)
        def mk_idx(toks):
            toks = np.concatenate(toks)
            L = len(toks)
            return np.ascontiguousarray(
                np.tile(toks.reshape(L // 16, 16).T.astype(np.int16), (8, 1)))
        return outs[0], outs[1], mk_idx(toksA), mk_idx(toksB)

    # layer-1 gather table: dinv*x in permuted row order, fp16
    xs = np.zeros((c.NSTAR, c.FIN), np.float32)
    xs[c.perm(np.arange(N))] = x * dinv[:, None]
    xs16 = np.ascontiguousarray(xs.astype(np.float16))

    xpad = np.zeros((c.NSTAR, c.FIN), np.float32)
    xpad[:N] = x

    def cat2(w, dt):
        w = np.asarray(w, dtype=np.float32)
        return np.ascontiguousarray(np.concatenate([w[0], w[1]], axis=1).astype(dt))

    w1cat = cat2(init_w1, np.float32)            # [128,128] f32
    w2cat = cat2(init_w2, np.float16)            # [64,128]  f16
    rw1c = cat2(0.5 * np.asarray(root_w1, np.float32), np.float16)
    rw2c = cat2(0.5 * np.asarray(root_w2, np.float32), np.float32)
    b1 = np.asarray(b1, dtype=np.float32)
    b2 = np.asarray(b2, dtype=np.float32)
    b1b = np.ascontiguousarray(
        np.tile(0.5 * np.concatenate([b1[0], b1[1]]), (P, 4)))
    b2b = np.ascontiguousarray(
        np.tile(0.5 * np.concatenate([b2[0], b2[1]]), (P, 4)))

    in_maps = []
    for cc in range(NC):
        base = cc * SHARD
        sA, sB, idxA, idxB = build_core(cc)
        dinvo = 0.5 * dinv_full[base:base + SHARD].reshape(c.W, P).T
        dinvt = dinv_full[base:base + SHARD].reshape(c.W, P).T
        in_maps.append({
            "xs": xs16,
            "xTow": np.ascontiguousarray(xpad[base:base + SHARD].T.astype(np.float16)),
            "w1cat": w1cat, "rw1c": rw1c, "w2cat": w2cat, "rw2c": rw2c,
            "b1b": b1b, "b2b": b2b,
            "dinvo": np.ascontiguousarray(dinvo.astype(np.float32)),
            "dinvt": np.ascontiguousarray(dinvt.astype(np.float32)),
            "idxA": idxA, "idxB": idxB,
            "sA": sA, "sB": sB,
        })
    return in_maps, NBA, NBB


# --------------------------------------------------------------------------
# device program
# --------------------------------------------------------------------------
def _build_program(c: Cfg, NBA, NBB):
    import concourse.tile as tile
    from concourse import bacc, mybir
    from concourse.masks import make_identity

    f32 = mybir.dt.float32
    f16 = mybir.dt.float16
    i16 = mybir.dt.int16
    fS = mybir.dt.float8e4 if c.SDT == "f8" else f16
    AL = mybir.AluOpType
    AF = mybir.ActivationFunctionType

    NBAtot, NBBtot = sum(NBA), sum(NBB)
    LA, LB = NBAtot * P, NBBtot * P

    nc = bacc.Bacc("TRN2", target_bir_lowering=False, debug=False,
                   num_devices=c.NC, num_swdge_queues=4)
    qrr = [0]

    def din(name, shape, dt=f32):
        return nc.dram_tensor(name, shape, dt, kind="ExternalInput")

    xs = din("xs", [c.NSTAR, 128], f16)          # layer-1 gather table
    xTow = din("xTow", [P, c.SHARD], f16)
    w1cat = din("w1cat", [P, 128], f32)
    rw1c = din("rw1c", [P, 128], f16)
    w2cat = din("w2cat", [64, 128], f16)
    rw2c = din("rw2c", [64, 128], f32)
    b1b = din("b1b", [P, 512]); b2b = din("b2b", [P, 512])
    dinvo = din("dinvo", [P, c.W])
    dinvt = din("dinvt", [P, c.W])
    idxA = din("idxA", [P, LA // 16], i16)
    idxB = din("idxB", [P, LB // 16], i16)
    sAd = din("sA", [P, NBAtot * P], fS)
    sBd = din("sB", [P, NBBtot * P], fS)
    yt = nc.dram_tensor("yt", [c.SHARD, 64], f32, kind="ExternalOutput")

    ccpad = nc.dram_tensor("ccpad", [c.SHARD, 128], f16)
    ccout = nc.dram_tensor("ccout", [c.NSTAR, 128], f16, addr_space="Shared")

    with tile.TileContext(nc) as tc, ExitStack() as ctx:
        cpool = ctx.enter_context(tc.tile_pool(name="consts", bufs=1))
        xtp = ctx.enter_context(tc.tile_pool(name="xtp", bufs=3))
        gth = ctx.enter_context(tc.tile_pool(name="gth", bufs=10))
        sgp = ctx.enter_context(tc.tile_pool(name="sgp", bufs=4))
        idxp = ctx.enter_context(tc.tile_pool(name="idxp", bufs=3))
        epi = ctx.enter_context(tc.tile_pool(name="epi", bufs=3))
        big = ctx.enter_context(tc.tile_pool(name="big", bufs=1))
        psA = ctx.enter_context(tc.tile_pool(name="psA", bufs=2, space="PSUM"))
        psB = ctx.enter_context(tc.tile_pool(name="psB", bufs=3, space="PSUM"))
        psC = ctx.enter_context(tc.tile_pool(name="psC", bufs=2, space="PSUM"))

        ident = cpool.tile([P, P], f32, tag="ident")
        make_identity(nc, ident[:])

        def load_const(dram, shape, tag, dt=f32):
            t = cpool.tile(shape, dt, tag=tag)
            nc.sync.dma_start(t[:], dram[:, :])
            return t

        w1_s = load_const(w1cat, [P, 128], "w1")
        rw1_s = load_const(rw1c, [P, 128], "rw1", f16)
        w2_s = load_const(w2cat, [64, 128], "w2", f16)
        rw2_s = load_const(rw2c, [64, 128], "rw2")
        b1_s = load_const(b1b, [P, 512], "b1")
        b2_s = load_const(b2b, [P, 512], "b2")
        dinvo_s = load_const(dinvo, [P, c.W], "dinvo")
        dinvt_s = load_const(dinvt, [P, c.W], "dinvt")

        # ---- queue warmup: tiny gather per SWDGE queue, overlaps prolog ----
        with nc.named_scope("warm"):
            wix = idxp.tile([P, 8], i16, tag="ixA")
            nc.sync.dma_start(wix[:], idxA[:, 0:8])
            for q in range(4):
                wg = gth.tile([P, 128], f16, tag="gath")
                nc.gpsimd.dma_gather(
                    out_ap=wg[:].rearrange("p (b f) -> p b f", f=128),
                    in_ap=xs[0:c.HALFA, :],
                    idxs_ap=wix[:, 0:8],
                    num_idxs=128, num_idxs_reg=128, elem_size=128,
                    single_packet=c.SP, queue_num=q)

        # ---- prolog: root1 (bias adds batched 4 windows per op) ----
        with nc.named_scope("prolog"):
            root1 = big.tile([P, c.SHARD], f32, tag="root")
            i = 0
            while i < c.W:
                n = min(8, c.W - i)
                xp = xtp.tile([P, 8 * 128], f16, tag="xtp")
                nc.sync.dma_start(xp[:, :n * 128], xTow[:, i * P:(i + n) * P])
                j = 0
                while j < n:
                    g = min(4, n - j)
                    ps = psA.tile([P, 512], f32, tag="grp")
                    for k in range(g):
                        nc.tensor.matmul(
                            out=ps[:, k * 128:(k + 1) * 128],
                            lhsT=xp[:, (j + k) * 128:(j + k + 1) * 128],
                            rhs=rw1_s[:], start=True, stop=True)
                    nc.vector.tensor_tensor(
                        out=root1[:, (i + j) * 128:(i + j + g) * 128],
                        in0=ps[:, :g * 128], in1=b1_s[:, :g * 128], op=AL.add)
                    j += g
                i += n

        def gather_call(tab_ap, ix_t, l0, nblk):
            g_t = gth.tile([P, nblk * 128], f16, tag="gath")
            nc.gpsimd.dma_gather(
                out_ap=g_t[:].rearrange("p (b f) -> p b f", f=128),
                in_ap=tab_ap,
                idxs_ap=ix_t[:, l0 // 16:(l0 + nblk * P) // 16],
                num_idxs=nblk * P, num_idxs_reg=nblk * P, elem_size=128,
                single_packet=c.SP, queue_num=qrr[0] % 4)
            qrr[0] += 1
            return g_t

        def s_load(sd, blk0, nblk):
            s_t = sgp.tile([P, nblk * 128], fS, tag="sg")
            nc.sync.dma_start(s_t[:], sd[:, blk0 * 128:(blk0 + nblk) * 128])
            return s_t

        # ---- layer 1: A+B interleaved per batch ----
        h1n = big.tile([P, c.W * 64], f16, tag="ht")
        pending = []      # (chunk_idx, issue_at_window)
        chunk_iter = iter(range(len(c.cc_chunks)))
        next_q = next(chunk_iter)

        def issue_cc(q):
            w0, w1 = c.cc_chunks[q]
            b0 = c.chunk_base[q]
            nc.gpsimd.collective_compute(
                "AllGather", AL.bypass,
                replica_groups=[list(range(c.NC))],
                ins=[ccpad[w0 * P:w1 * P, :].opt()],
                outs=[ccout[b0:b0 + c.NC * (w1 - w0) * P, :].opt()])

        def on_window1(w):
            nonlocal next_q
            while pending and w >= pending[0][1]:
                issue_cc(pending.pop(0)[0])
            sc = epi.tile([P, 128], f16, tag="sc")
            nc.vector.memset(sc[:, 64:], 0.0)
            nc.scalar.mul(sc[:, :64], h1n[:, w * 64:(w + 1) * 64],
                          dinvt_s[:, w:w + 1])
            nc.sync.dma_start(
                ccpad[w * P:(w + 1) * P, :]
                .rearrange("(k p) f -> p k f", p=P),
                sc[:].rearrange("p (k f) -> p k f", k=1))
            if next_q is not None and w == c.cc_chunks[next_q][1] - 1:
                pending.append((next_q, w + c.WB))
                next_q = next(chunk_iter, None)

        with nc.named_scope("layer1"):
            blkA = blkB = 0
            tokA = tokB = 0
            for sb in c.sbatches:
                sbA = sum(NBA[w] for b in sb for w in b) * P
                sbB = sum(NBB[w] for b in sb for w in b) * P
                ixA = idxp.tile([P, sbA // 16], i16, tag="ixA")
                nc.sync.dma_start(ixA[:], idxA[:, tokA // 16:(tokA + sbA) // 16])
                ixB = idxp.tile([P, sbB // 16], i16, tag="ixB")
                nc.sync.dma_start(ixB[:], idxB[:, tokB // 16:(tokB + sbB) // 16])
                lA = lB = 0
                for batch in sb:
                    nA = sum(NBA[w] for w in batch)
                    nB = sum(NBB[w] for w in batch)
                    gA = gather_call(xs[0:c.HALFA, :], ixA, lA, nA)
                    gB = gather_call(xs[c.HALFA:c.NSTAR, :], ixB, lB, nB)
                    sA_t = s_load(sAd, blkA, nA)
                    sB_t = s_load(sBd, blkB, nB)
                    lA += nA * P
                    lB += nB * P
                    oA = oB = 0
                    for w in batch:
                        pw = psB.tile([P, 128], f32, tag="pw")
                        nmm = NBA[w] + NBB[w]
                        k = 0
                        for j in range(NBA[w]):
                            b = oA + j
                            nc.tensor.matmul(
                                out=pw[:],
                                lhsT=gA[:, b * 128:(b + 1) * 128],
                                rhs=sA_t[:, b * 128:(b + 1) * 128],
                                start=(k == 0), stop=(k == nmm - 1))
                            k += 1
                        for j in range(NBB[w]):
                            b = oB + j
                            nc.tensor.matmul(
                                out=pw[:],
                                lhsT=gB[:, b * 128:(b + 1) * 128],
                                rhs=sB_t[:, b * 128:(b + 1) * 128],
                                start=(k == 0), stop=(k == nmm - 1))
                            k += 1
                        oA += NBA[w]; oB += NBB[w]
                        # window transform: agg.T is already lhsT-oriented
                        utc = epi.tile([P, 128], f32, tag="utc")
                        nc.scalar.copy(utc[:], pw[:])
                        pw2 = psC.tile([P, 128], f32, tag="pw2")
                        nc.tensor.matmul(out=pw2[:], lhsT=utc[:],
                                         rhs=w1_s[:], start=True, stop=True)
                        t2 = epi.tile([P, 128], f32, tag="t2")
                        nc.vector.scalar_tensor_tensor(
                            out=t2[:], in0=pw2[:], scalar=dinvo_s[:, w:w + 1],
                            in1=root1[:, w * 128:(w + 1) * 128],
                            op0=AL.mult, op1=AL.add)
                        t3 = epi.tile([P, 128], f32, tag="t3")
                        nc.scalar.activation(t3[:], t2[:], AF.Relu)
                        nc.vector.tensor_tensor(
                            out=h1n[:, w * 64:(w + 1) * 64],
                            in0=t3[:, :64], in1=t3[:, 64:], op=AL.add)
                        on_window1(w)
                    blkA += nA; blkB += nB
                tokA += sbA; tokB += sbB

        with nc.named_scope("cc"):
            for q, _ in pending:
                issue_cc(q)

        # ---- layer 2 pass A: aggregate rows < B0, bank aggA.T; root2 ----
        aggA16 = big.tile([64, c.W * 128], f16, tag="aggA")
        root2 = big.tile([P, c.SHARD], f32, tag="root")  # aliases root1

        def root2_group(j, g):
            ps = psA.tile([P, 512], f32, tag="grp")
            for k in range(g):
                u2 = epi.tile([P, 64], f32, tag="u2")
                nc.scalar.copy(u2[:], h1n[:, (j + k) * 64:(j + k + 1) * 64])
                tp_ = psC.tile([P, 128], f32, tag="pw2")
                nc.tensor.transpose(out=tp_[:64, :], in_=u2[:],
                                    identity=ident[:])
                hl = epi.tile([64, 128], f32, tag="hl")
                nc.scalar.copy(hl[:], tp_[:64, :])
                nc.tensor.matmul(out=ps[:, k * 128:(k + 1) * 128],
                                 lhsT=hl[:], rhs=rw2_s[:],
                                 start=True, stop=True)
            nc.vector.tensor_tensor(
                out=root2[:, j * 128:(j + g) * 128],
                in0=ps[:, :g * 128], in1=b2_s[:, :g * 128], op=AL.add)

        with nc.named_scope("l2passA"):
            blkA = 0
            tokA = 0
            r2done = 0
            for sb in c.sbatches:
                sbA = sum(NBA[w] for b in sb for w in b) * P
                ixA = idxp.tile([P, sbA // 16], i16, tag="ixA")
                nc.sync.dma_start(ixA[:], idxA[:, tokA // 16:(tokA + sbA) // 16])
                lA = 0
                for batch in sb:
                    nA = sum(NBA[w] for w in batch)
                    gA = gather_call(ccout[0:c.HALFA, :], ixA, lA, nA)
                    sA_t = s_load(sAd, blkA, nA)
                    lA += nA * P
                    oA = 0
                    for w in batch:
                        pw = psB.tile([P, 128], f32, tag="pw")
                        for j in range(NBA[w]):
                            b = oA + j
                            nc.tensor.matmul(
                                out=pw[:],
                                lhsT=gA[:, b * 128:(b + 1) * 128],
                                rhs=sA_t[:, b * 128:(b + 1) * 128],
                                start=(j == 0), stop=(j == NBA[w] - 1))
                        oA += NBA[w]
                        nc.scalar.copy(aggA16[:, w * 128:(w + 1) * 128],
                                       pw[:64, :])
                        # interleave root2 construction (needs only h1n)
                        while r2done <= w - 3:
                            g = min(4, c.W - r2done)
                            root2_group(r2done, g)
                            r2done += g
                    blkA += nA
                tokA += sbA
            while r2done < c.W:
                g = min(4, c.W - r2done)
                root2_group(r2done, g)
                r2done += g

        # ---- layer 2 pass B: aggregate rows >= B0, combine + transform ----
        yn = big.tile([P, c.W * 64], f32, tag="yt")
        with nc.named_scope("l2passB"):
            blkB = 0
            tokB = 0
            for sb in c.sbatches:
                sbB = sum(NBB[w] for b in sb for w in b) * P
                ixB = idxp.tile([P, sbB // 16], i16, tag="ixB")
                nc.sync.dma_start(ixB[:], idxB[:, tokB // 16:(tokB + sbB) // 16])
                lB = 0
                for batch in sb:
                    nB = sum(NBB[w] for w in batch)
                    gB = gather_call(ccout[c.HALFA:c.NSTAR, :], ixB, lB, nB)
                    sB_t = s_load(sBd, blkB, nB)
                    lB += nB * P
                    oB = 0
                    for w in batch:
                        pw = psB.tile([P, 128], f32, tag="pw")
                        for j in range(NBB[w]):
                            b = oB + j
                            nc.tensor.matmul(
                                out=pw[:],
                                lhsT=gB[:, b * 128:(b + 1) * 128],
                                rhs=sB_t[:, b * 128:(b + 1) * 128],
                                start=(j == 0), stop=(j == NBB[w] - 1))
                        oB += NBB[w]
                        utcB = epi.tile([64, 128], f16, tag="utcB")
                        nc.scalar.copy(utcB[:], pw[:64, :])
                        pw2 = psC.tile([P, 128], f32, tag="pw2")
                        nc.tensor.matmul(
                            out=pw2[:], lhsT=aggA16[:, w * 128:(w + 1) * 128],
                            rhs=w2_s[:], start=True, stop=False)
                        nc.tensor.matmul(
                            out=pw2[:], lhsT=utcB[:],
                            rhs=w2_s[:], start=False, stop=True)
                        t2 = epi.tile([P, 128], f32, tag="t2")
                        nc.vector.scalar_tensor_tensor(
                            out=t2[:], in0=pw2[:], scalar=dinvo_s[:, w:w + 1],
                            in1=root2[:, w * 128:(w + 1) * 128],
                            op0=AL.mult, op1=AL.add)
                        t3 = epi.tile([P, 128], f32, tag="t3")
                        nc.scalar.activation(t3[:], t2[:], AF.Relu)
                        nc.vector.tensor_tensor(
                            out=yn[:, w * 64:(w + 1) * 64],
                            in0=t3[:, :64], in1=t3[:, 64:], op=AL.add)
                    blkB += nB
                tokB += sbB
        nc.sync.dma_start(yt[:, :].rearrange("(w p) f -> p w f", p=P), yn[:])

    nc.compile()
    return nc


_cache = {}


def prepare(inputs, cfg_kw=None):
    c = Cfg(**(cfg_kw or REAL))
    in_maps, NBA, NBB = _preprocess(c, **inputs)
    key = (tuple(sorted((cfg_kw or REAL).items())), tuple(NBA), tuple(NBB))
    if key not in _cache:
        _cache[key] = _build_program(c, NBA, NBB)
    return c, _cache[key], in_maps


def kernel(x, edge_index, init_w1, root_w1, b1, init_w2, root_w2, b2,
           _trace=False, _cfg=None):
    from concourse import bass_utils
    inputs = dict(x=np.asarray(x), edge_index=np.asarray(edge_index),
                  init_w1=np.asarray(init_w1), root_w1=np.asarray(root_w1),
                  b1=np.asarray(b1), init_w2=np.asarray(init_w2),
                  root_w2=np.asarray(root_w2), b2=np.asarray(b2))
    c, nc, in_maps = prepare(inputs, _cfg)
    res = bass_utils.run_bass_kernel_spmd(
        nc, in_maps, core_ids=list(range(c.NC)), trace=_trace)
    out = np.concatenate([res.results[cc]["yt"] for cc in range(c.NC)],
                         axis=0)[:c.N]
    if _trace:
        kernel._last = res
    return np.ascontiguousarray(out.astype(np.float32))
